# revision 20
# baseline (speedup 1.0000x reference)
"""Multi-head self-attention (B=8, T=2048, C=192, H=6, HS=32) on 8 TRN2 cores.

Data-parallel over batch: core i computes batch element i fully on-chip.

Design (driven by the CoreSim cost model, which charges a matmul only its
streamed output columns):
  qT/kT [d,t] kept fp32 (float32r matmuls: 1 cyc/row at N>=512) - exact scores.
  S^T [s,t] tiles per head pair -> exp split across ACT (exact) / DVE / GpSimd
  (Schraudolph int16 bit-trick writing bf16 bit patterns directly).
  AV flipped: O[t,d] = P^T[s,t-tile].T @ [v_h | 1]  (N=33 streamed cols; the
  ones column accumulates the softmax denominator r as col 32).
  Normalize with r on partitions (reciprocal + one broadcast multiply), PE
  transpose [t,d]->[d,t], then the output projection with the bias folded in
  as a ones row of otn_b.
"""

import numpy as np
import ml_dtypes
from contextlib import ExitStack

import concourse.bass as bass
import concourse.tile as tile
from concourse import bacc, mybir
from concourse.bass import broadcast_tensor_aps
from concourse.bass_utils import run_bass_kernel_spmd

B, T, C = 8, 2048, 192
H, HS = 6, 32
P = 128
TCH = 512            # t-chunk per tc0 block
NT = T // TCH        # 4
NS = T // P          # 16 s-tiles
NJ = TCH // P        # 4 t-subtiles per chunk
SCALE = 1.0 / float(np.sqrt(HS))
BF16 = mybir.dt.bfloat16
F32 = mybir.dt.float32
F32R = mybir.dt.float32r
I16 = mybir.dt.int16
Exp = mybir.ActivationFunctionType.Exp

# Schraudolph constants for bf16-domain exp: bits = int16(s*EXP_A + EXP_B),
# reinterpreted as bf16 ~= exp(s*SCALE).
EXP_A = SCALE * 128.0 / float(np.log(2.0))
EXP_B = 127.0 * 128.0 - 486411.0 / 65536.0 + 0.5

_CACHE = {}

# engine assignment for exp tiles: index = si*3+p (48 per tc0)
# A=ACT exact exp, D=DVE Schraudolph. GpSimd cannot access PSUM.
N_ACT = 25  # of 48 per tc0


def _exp_engine(idx):
    if idx < 6:
        return "A"
    if idx >= 45:
        return "D"
    return "A" if (idx * N_ACT) // 48 != ((idx + 1) * N_ACT) // 48 else "D"


_EXPPAT = "".join(_exp_engine(i) for i in range(48))


def build_nc():
    nc = bacc.Bacc()
    xT = nc.declare_dram_parameter("xT", [C, T], F32R, isOutput=False)
    wq = nc.declare_dram_parameter("wq", [C, H * HS], F32R, isOutput=False)
    wk = nc.declare_dram_parameter("wk", [C, H * HS], F32R, isOutput=False)
    x16 = nc.declare_dram_parameter("x16", [C, T], BF16, isOutput=False)
    wv = nc.declare_dram_parameter("wv", [C, H * HS], BF16, isOutput=False)
    wpa = nc.declare_dram_parameter("wpa", [P, C], BF16, isOutput=False)
    wpb = nc.declare_dram_parameter("wpb", [C - P + 1, C], BF16, isOutput=False)
    ident = nc.declare_dram_parameter("ident", [P, P], BF16, isOutput=False)
    out = nc.declare_dram_parameter("out", [T, C], F32, isOutput=True)

    with tile.TileContext(nc) as tc, ExitStack() as ctx:
        singles = ctx.enter_context(tc.tile_pool(name="singles", bufs=1))
        vpool = ctx.enter_context(tc.tile_pool(name="vpool", bufs=1))
        ptp_pool = ctx.enter_context(tc.tile_pool(name="ptp", bufs=8))
        stage_pool = ctx.enter_context(tc.tile_pool(name="stage", bufs=2))
        rr_pool = ctx.enter_context(tc.tile_pool(name="rr", bufs=2))
        ysb_pool = ctx.enter_context(tc.tile_pool(name="ysb", bufs=3))

        # ---------------- input DMA ----------------
        xT_a = singles.tile([P, T], F32R)
        xT_b = singles.tile([C - P, T], F32R)
        x16_a = singles.tile([P, T], BF16)
        x16_b = singles.tile([C - P, T], BF16)
        wq_a = singles.tile([P, H * HS], F32R)
        wq_b = singles.tile([C - P, H * HS], F32R)
        wk_a = singles.tile([P, H * HS], F32R)
        wk_b = singles.tile([C - P, H * HS], F32R)
        wv_a = singles.tile([P, H * HS], BF16)
        wv_b = singles.tile([C - P, H * HS], BF16)
        nc.gpsimd.dma_start(wq_a, wq[0:P, :])
        nc.gpsimd.dma_start(wq_b, wq[P:C, :])
        nc.gpsimd.dma_start(wk_a, wk[0:P, :])
        nc.gpsimd.dma_start(wk_b, wk[P:C, :])
        nc.gpsimd.dma_start(wv_a, wv[0:P, :])
        nc.gpsimd.dma_start(wv_b, wv[P:C, :])
        for t0 in range(0, T, TCH):
            nc.sync.dma_start(xT_a[:, t0:t0 + TCH], xT[0:P, t0:t0 + TCH])
            nc.sync.dma_start(xT_b[:, t0:t0 + TCH], xT[P:C, t0:t0 + TCH])
            nc.gpsimd.dma_start(x16_a[:, t0:t0 + TCH], x16[0:P, t0:t0 + TCH])
            nc.gpsimd.dma_start(x16_b[:, t0:t0 + TCH], x16[P:C, t0:t0 + TCH])
        wpa_sb = singles.tile([P, C], BF16)
        wpb_sb = singles.tile([C - P + 1, C], BF16)
        id_sb = singles.tile([P, P], BF16)
        nc.sync.dma_start(wpa_sb, wpa[:, :])
        nc.sync.dma_start(wpb_sb, wpb[:, :])
        nc.sync.dma_start(id_sb, ident[:, :])

        qT_a = singles.tile([P, T], F32R)
        qT_b = singles.tile([C - P, T], F32R)
        kT_a = singles.tile([P, T], F32R)
        kT_b = singles.tile([C - P, T], F32R)
        otn_a = singles.tile([P, T], BF16)
        otn_b = singles.tile([C - P + 1, T], BF16)
        # ones row for the bias trick in the output projection
        nc.gpsimd.memset(otn_b[C - P:C - P + 1, :], 1.0)

        v33 = []
        for si in range(NS):
            v33.append(vpool.tile([P, H * 33], BF16, name=f"v33_{si}"))

        # ---------------- phase 2 ----------------
        def hsrc(h):
            if h < 4:
                return kT_a, qT_a, HS * h
            return kT_b, qT_b, HS * (h - 4)

        with (
            tc.tile_pool(name="pst", bufs=3, space="PSUM") as pst_pool,
            tc.tile_pool(name="pav", bufs=1, space="PSUM") as pav_pool,
        ):
            state = {}
            PYOFF = (0, 192, 512, 704)
            VOFF = (0, 192, 512, 704)

            def phase1_block(g):
                t0 = g * TCH
                for name, wa, wb, dsta, dstb in (
                    ("q", wq_a, wq_b, qT_a, qT_b),
                    ("k", wk_a, wk_b, kT_a, kT_b),
                ):
                    slot = pst_pool.tile([P, 2 * TCH], F32, name=f"p1{name}",
                                         tag="stp")
                    for idx, (dlo, dsz, dst) in enumerate(
                            ((0, P, dsta), (P, C - P, dstb))):
                        ps = slot[:, idx * TCH:(idx + 1) * TCH]
                        nc.tensor.matmul(
                            ps[0:dsz, :], wa[:, dlo:dlo + dsz],
                            xT_a[:, t0:t0 + TCH], start=True, stop=False)
                        nc.tensor.matmul(
                            ps[0:dsz, :], wb[:, dlo:dlo + dsz],
                            xT_b[:, t0:t0 + TCH], start=False, stop=True)
                        if name == "q":
                            nc.scalar.copy(dst[0:dsz, t0:t0 + TCH],
                                           ps[0:dsz, :])
                        else:
                            nc.vector.tensor_copy(
                                dst[0:dsz, t0:t0 + TCH], ps[0:dsz, :])
                slot = pst_pool.tile([P, 2 * TCH], F32, name="p1v", tag="stp")
                for i in range(NJ):
                    si = g * NJ + i
                    s0 = si * P
                    ps = slot[:, VOFF[i]:VOFF[i] + H * HS]
                    nc.tensor.matmul(ps, x16_a[:, s0:s0 + P],
                                     wv_a, start=True, stop=False)
                    nc.tensor.matmul(ps, x16_b[:, s0:s0 + P],
                                     wv_b, start=False, stop=True)
                    va_r = v33[si].rearrange("p (h e) -> p h e", h=H)
                    ps_r = ps.rearrange("p (h d) -> p h d", h=H)
                    nc.vector.tensor_copy(va_r[:, :, 0:HS], ps_r)
                    nc.gpsimd.memset(va_r[:, :, HS:HS + 1], 1.0)

            def emit_qkt(tci, si, p):
                tc0 = tci * TCH
                s0 = si * P
                stp = pst_pool.tile([P, 2 * TCH], F32, name="stp", tag="stp")
                for half in (0, 1):
                    h = 2 * p + half
                    kT_t, qT_t, pb = hsrc(h)
                    nc.tensor.matmul(
                        stp[:, half * TCH:(half + 1) * TCH],
                        kT_t[pb:pb + HS, s0:s0 + P],
                        qT_t[pb:pb + HS, tc0:tc0 + TCH],
                        start=True, stop=True, tile_position=(pb, 0))
                return stp

            def emit_exp(tci, si, p, stp):
                eng = _EXPPAT[si * 3 + p]
                ptp = ptp_pool.tile([P, 2 * TCH], BF16, name="ptp", tag="ptp")
                if eng == "A":
                    nc.scalar.activation(ptp, stp, Exp, scale=SCALE)
                else:
                    nc.vector.tensor_scalar(
                        ptp.bitcast(I16), stp, EXP_A, EXP_B,
                        mybir.AluOpType.mult, mybir.AluOpType.add)
                return ptp

            def emit_av(tci, si, ptps):
                av = state["av"]
                for h in range(H):
                    bank, hl = divmod(h, 3)
                    ptp = ptps[h // 2]
                    half = h % 2
                    for j in range(NJ):
                        off = hl * 132 + j * 33
                        nc.tensor.matmul(
                            av[bank][:, off:off + 33],
                            ptp[:, half * TCH + j * P:half * TCH + (j + 1) * P],
                            v33[si][:, h * 33:(h + 1) * 33],
                            start=(si == 0 and hl == 0 and j == 0),
                            stop=(si == NS - 1 and hl == 2 and j == NJ - 1),
                            skip_group_check=True)

            def tail_norm(tci):
                av = state["av"]
                sa = stage_pool.tile([P, NJ * P], BF16, name="sa", tag="sa")
                sb = stage_pool.tile([P, NJ * (C - P)], BF16,
                                     name="sb", tag="sb")
                sa_v = sa.rearrange("p (j h e) -> p h j e", j=NJ, h=4)
                sb_v = sb.rearrange("p (j h e) -> p h j e", j=NJ, h=2)
                outs = []
                for bank in range(2):
                    rr = rr_pool.tile([P, 12], F32, name=f"rr{bank}",
                                      tag=f"rr{bank}")
                    av_v = av[bank][:, 0:396].rearrange(
                        "p (hl j e) -> p hl j e", hl=3, j=NJ)
                    rr_v = rr.rearrange("p (hl j e) -> p hl j e", hl=3, j=NJ)
                    nc.vector.reciprocal(rr_v, av_v[:, :, :, 32:33])
                    outs.append((av_v[:, :, :, 0:HS], rr_v))
                (o1, r1), (o2, r2) = outs
                for dst, src_, rsrc in (
                    (sa_v[:, 0:3], o1, r1),
                    (sa_v[:, 3:4], o2[:, 0:1], r2[:, 0:1]),
                    (sb_v[:, 0:2], o2[:, 1:3], r2[:, 1:3]),
                ):
                    s_b, r_b = broadcast_tensor_aps(src_, rsrc)
                    nc.vector.tensor_tensor(dst, s_b, r_b,
                                            mybir.AluOpType.mult)
                state["stage"] = (sa, sb)

            def tail_transpose(tci):
                sa, sb = state["stage"]
                slot = pst_pool.tile([P, 2 * TCH], F32, name="pot", tag="stp")
                pot = slot[:, 0:TCH].bitcast(BF16)
                for j in range(NJ):
                    nc.tensor.transpose(
                        pot[:, j * 2 * P:j * 2 * P + P],
                        sa[:, j * P:(j + 1) * P], id_sb)
                    nc.tensor.transpose(
                        pot[0:C - P, j * 2 * P + P:(j + 1) * 2 * P],
                        sb[:, j * (C - P):(j + 1) * (C - P)], id_sb)
                state["pot"] = pot

            def tail_otcopy(tci):
                pot = state["pot"]
                tc0 = tci * TCH
                pot_v = pot.rearrange("p (j two t) -> p j two t", j=NJ, two=2)
                dst_a = otn_a[:, tc0:tc0 + TCH].rearrange(
                    "p (j t) -> p j t", j=NJ)
                dst_b = otn_b[0:C - P, tc0:tc0 + TCH].rearrange(
                    "p (j t) -> p j t", j=NJ)
                nc.vector.tensor_copy(dst_a, pot_v[:, :, 0, :])
                nc.vector.tensor_copy(dst_b, pot_v[0:C - P, :, 1, :])

            def tail_py_alloc():
                state["py"] = pst_pool.tile([P, 2 * TCH], F32, name="pys",
                                            tag="stp")

            def tail_proj(tci, j):
                tt = tci * TCH + j * P
                py = state["py"][:, PYOFF[j]:PYOFF[j] + C]
                nc.tensor.matmul(py, otn_a[:, tt:tt + P], wpa_sb,
                                 start=True, stop=False)
                nc.tensor.matmul(py, otn_b[:, tt:tt + P], wpb_sb,
                                 start=False, stop=True)
                ysb = ysb_pool.tile([P, C], F32, name="ysb", tag="ysb")
                nc.vector.tensor_copy(ysb, py)
                nc.sync.dma_start(out[tt:tt + P, :], ysb)

            prev = None
            for tci in range(NT):
                av = [pav_pool.tile([P, TCH], F32, name=f"av{b}",
                                    tag=f"av{b}") for b in range(2)]
                state["av"] = av
                pend = None
                for si in range(NS):
                    if tci == 0 and si in (0, 1):
                        for g in (range(2) if si == 0 else range(2, 4)):
                            phase1_block(g)
                    ptps = []
                    for p in range(3):
                        if p == 2:
                            if pend is not None:
                                emit_av(tci, si - 1, pend)
                                pend = None
                        stp = emit_qkt(tci, si, p)
                        ptps.append(emit_exp(tci, si, p, stp))
                        if prev is not None and si == 1 and p == 1:
                            tail_transpose(prev)
                    if prev is not None:
                        if si == 1:
                            tail_otcopy(prev)
                        elif si == 2:
                            tail_py_alloc()
                            tail_proj(prev, 0)
                            tail_proj(prev, 1)
                        elif si == 3:
                            tail_proj(prev, 2)
                            tail_proj(prev, 3)
                    pend = ptps
                emit_av(tci, NS - 1, pend)
                tail_norm(tci)
                prev = tci
            sa, sb = state["stage"]
            slot = pst_pool.tile([P, 2 * TCH], F32, name="potf", tag="stp")
            potf = slot[:, 0:TCH].bitcast(BF16)
            tail_py_alloc()
            tc0f = prev * TCH
            for j in range(NJ):
                nc.tensor.transpose(
                    potf[:, j * 2 * P:j * 2 * P + P],
                    sa[:, j * P:(j + 1) * P], id_sb)
                nc.tensor.transpose(
                    potf[0:C - P, j * 2 * P + P:(j + 1) * 2 * P],
                    sb[:, j * (C - P):(j + 1) * (C - P)], id_sb)
                tt = tc0f + j * P
                eng = nc.vector if j % 2 == 0 else nc.scalar
                if j % 2 == 0:
                    nc.vector.tensor_copy(otn_a[:, tt:tt + P],
                                          potf[:, j * 2 * P:j * 2 * P + P])
                    nc.vector.tensor_copy(
                        otn_b[0:C - P, tt:tt + P],
                        potf[0:C - P, j * 2 * P + P:(j + 1) * 2 * P])
                else:
                    nc.scalar.copy(otn_a[:, tt:tt + P],
                                   potf[:, j * 2 * P:j * 2 * P + P])
                    nc.scalar.copy(
                        otn_b[0:C - P, tt:tt + P],
                        potf[0:C - P, j * 2 * P + P:(j + 1) * 2 * P])
                py = state["py"][:, PYOFF[j]:PYOFF[j] + C]
                nc.tensor.matmul(py, otn_a[:, tt:tt + P], wpa_sb,
                                 start=True, stop=False)
                nc.tensor.matmul(py, otn_b[:, tt:tt + P], wpb_sb,
                                 start=False, stop=True)
                ysb = ysb_pool.tile([P, C], F32, name="ysbf", tag="ysb")
                if j % 2 == 0:
                    nc.scalar.copy(ysb, py)
                else:
                    nc.vector.tensor_copy(ysb, py)
                nc.sync.dma_start(out[tt:tt + P, :], ysb)

    nc.compile()
    return nc


def _get_nc():
    if "nc" not in _CACHE:
        _CACHE["nc"] = build_nc()
    return _CACHE["nc"]


def make_in_maps(x, Wq, Wk, Wv, Wproj, bproj):
    bf = ml_dtypes.bfloat16
    x = np.asarray(x, np.float32)
    pack32 = lambda w: np.ascontiguousarray(
        np.transpose(np.asarray(w, np.float32), (1, 0, 2)).reshape(C, H * HS))
    wq_, wk_ = pack32(Wq), pack32(Wk)
    wv_ = pack32(Wv).astype(bf)
    wp = np.asarray(Wproj, np.float32)
    wpa_ = np.ascontiguousarray(wp[0:P, :]).astype(bf)
    wpb_ = np.concatenate(
        [wp[P:, :], np.asarray(bproj, np.float32).reshape(1, C)], axis=0
    ).astype(bf)
    ident = np.eye(P, dtype=np.float32).astype(bf)
    maps = []
    for i in range(B):
        xti = np.ascontiguousarray(x[i].T)
        maps.append({"xT": xti, "x16": xti.astype(bf), "wq": wq_, "wk": wk_,
                     "wv": wv_, "wpa": wpa_, "wpb": wpb_, "ident": ident})
    return maps


def run(inputs, trace=False, **kw):
    nc = _get_nc()
    in_maps = make_in_maps(**inputs)
    res = run_bass_kernel_spmd(nc, in_maps, core_ids=list(range(B)),
                               trace=trace, **kw)
    y = np.stack([np.asarray(res.results[i]["out"], np.float32)
                  for i in range(B)], axis=0)
    return y, res


def kernel(**inputs):
    y, _ = run(inputs, trace=False)
    return y


# revision 21
# speedup vs baseline: 1.0049x; 1.0049x over previous
"""Multi-head self-attention (B=8, T=2048, C=192, H=6, HS=32) on 8 TRN2 cores.

Data-parallel over batch: core i computes batch element i fully on-chip.

Design (driven by the CoreSim cost model, which charges a matmul only its
streamed output columns):
  qT/kT [d,t] kept fp32 (float32r matmuls: 1 cyc/row at N>=512) - exact scores.
  S^T [s,t] tiles per head pair -> exp split across ACT (exact) / DVE / GpSimd
  (Schraudolph int16 bit-trick writing bf16 bit patterns directly).
  AV flipped: O[t,d] = P^T[s,t-tile].T @ [v_h | 1]  (N=33 streamed cols; the
  ones column accumulates the softmax denominator r as col 32).
  Normalize with r on partitions (reciprocal + one broadcast multiply), PE
  transpose [t,d]->[d,t], then the output projection with the bias folded in
  as a ones row of otn_b.
"""

import numpy as np
import ml_dtypes
from contextlib import ExitStack

import concourse.bass as bass
import concourse.tile as tile
from concourse import bacc, mybir
from concourse.bass import broadcast_tensor_aps
from concourse.bass_utils import run_bass_kernel_spmd

B, T, C = 8, 2048, 192
H, HS = 6, 32
P = 128
TCH = 512            # t-chunk per tc0 block
NT = T // TCH        # 4
NS = T // P          # 16 s-tiles
NJ = TCH // P        # 4 t-subtiles per chunk
SCALE = 1.0 / float(np.sqrt(HS))
BF16 = mybir.dt.bfloat16
F32 = mybir.dt.float32
F32R = mybir.dt.float32r
I16 = mybir.dt.int16
Exp = mybir.ActivationFunctionType.Exp

# Schraudolph constants for bf16-domain exp: bits = int16(s*EXP_A + EXP_B),
# reinterpreted as bf16 ~= exp(s*SCALE).
EXP_A = SCALE * 128.0 / float(np.log(2.0))
EXP_B = 127.0 * 128.0 - 486411.0 / 65536.0 + 0.5

_CACHE = {}

# engine assignment for exp tiles: index = si*3+p (48 per tc0)
# A=ACT exact exp, D=DVE Schraudolph. GpSimd cannot access PSUM.
N_ACT = 25  # of 48 per tc0


def _exp_engine(idx):
    if idx < 6:
        return "A"
    if idx >= 45:
        return "D"
    return "A" if (idx * N_ACT) // 48 != ((idx + 1) * N_ACT) // 48 else "D"


_EXPPAT = "".join(_exp_engine(i) for i in range(48))


def build_nc():
    nc = bacc.Bacc()
    xT = nc.declare_dram_parameter("xT", [C, T], F32R, isOutput=False)
    wq = nc.declare_dram_parameter("wq", [C, H * HS], F32R, isOutput=False)
    wk = nc.declare_dram_parameter("wk", [C, H * HS], F32R, isOutput=False)
    x16 = nc.declare_dram_parameter("x16", [C, T], BF16, isOutput=False)
    wv = nc.declare_dram_parameter("wv", [C, H * HS], BF16, isOutput=False)
    wpa = nc.declare_dram_parameter("wpa", [P, C], BF16, isOutput=False)
    wpb = nc.declare_dram_parameter("wpb", [C - P + 1, C], BF16, isOutput=False)
    ident = nc.declare_dram_parameter("ident", [P, P], BF16, isOutput=False)
    out = nc.declare_dram_parameter("out", [T, C], F32, isOutput=True)

    with tile.TileContext(nc) as tc, ExitStack() as ctx:
        singles = ctx.enter_context(tc.tile_pool(name="singles", bufs=1))
        vpool = ctx.enter_context(tc.tile_pool(name="vpool", bufs=1))
        ptp_pool = ctx.enter_context(tc.tile_pool(name="ptp", bufs=8))
        stage_pool = ctx.enter_context(tc.tile_pool(name="stage", bufs=2))
        rr_pool = ctx.enter_context(tc.tile_pool(name="rr", bufs=2))
        ysb_pool = ctx.enter_context(tc.tile_pool(name="ysb", bufs=3))

        # ---------------- input DMA ----------------
        xT_a = singles.tile([P, T], F32R)
        xT_b = singles.tile([C - P, T], F32R)
        x16_a = singles.tile([P, T], BF16)
        x16_b = singles.tile([C - P, T], BF16)
        wq_a = singles.tile([P, H * HS], F32R)
        wq_b = singles.tile([C - P, H * HS], F32R)
        wk_a = singles.tile([P, H * HS], F32R)
        wk_b = singles.tile([C - P, H * HS], F32R)
        wv_a = singles.tile([P, H * HS], BF16)
        wv_b = singles.tile([C - P, H * HS], BF16)
        nc.gpsimd.dma_start(wq_a, wq[0:P, :])
        nc.gpsimd.dma_start(wq_b, wq[P:C, :])
        nc.gpsimd.dma_start(wk_a, wk[0:P, :])
        nc.gpsimd.dma_start(wk_b, wk[P:C, :])
        nc.gpsimd.dma_start(wv_a, wv[0:P, :])
        nc.gpsimd.dma_start(wv_b, wv[P:C, :])
        for t0 in range(0, T, TCH):
            nc.sync.dma_start(xT_a[:, t0:t0 + TCH], xT[0:P, t0:t0 + TCH])
            nc.sync.dma_start(xT_b[:, t0:t0 + TCH], xT[P:C, t0:t0 + TCH])
            nc.gpsimd.dma_start(x16_a[:, t0:t0 + TCH], x16[0:P, t0:t0 + TCH])
            nc.gpsimd.dma_start(x16_b[:, t0:t0 + TCH], x16[P:C, t0:t0 + TCH])
        wpa_sb = singles.tile([P, C], BF16)
        wpb_sb = singles.tile([C - P + 1, C], BF16)
        id_sb = singles.tile([P, P], BF16)
        nc.sync.dma_start(wpa_sb, wpa[:, :])
        nc.sync.dma_start(wpb_sb, wpb[:, :])
        nc.sync.dma_start(id_sb, ident[:, :])

        qT_a = singles.tile([P, T], F32R)
        qT_b = singles.tile([C - P, T], F32R)
        kT_a = singles.tile([P, T], F32R)
        kT_b = singles.tile([C - P, T], F32R)
        otn_a = singles.tile([P, T], BF16)
        otn_b = singles.tile([C - P + 1, T], BF16)
        # ones row for the bias trick in the output projection
        nc.gpsimd.memset(otn_b[C - P:C - P + 1, :], 1.0)

        v33 = []
        for si in range(NS):
            v33.append(vpool.tile([P, H * 33], BF16, name=f"v33_{si}"))

        # ---------------- phase 2 ----------------
        def hsrc(h):
            if h < 4:
                return kT_a, qT_a, HS * h
            return kT_b, qT_b, HS * (h - 4)

        with (
            tc.tile_pool(name="pst", bufs=3, space="PSUM") as pst_pool,
            tc.tile_pool(name="pav", bufs=1, space="PSUM") as pav_pool,
        ):
            state = {}
            PYOFF = (0, 192, 512, 704)
            VOFF = (0, 192, 512, 704)

            def phase1_block(g):
                t0 = g * TCH
                for name, wa, wb, dsta, dstb in (
                    ("q", wq_a, wq_b, qT_a, qT_b),
                    ("k", wk_a, wk_b, kT_a, kT_b),
                ):
                    slot = pst_pool.tile([P, 2 * TCH], F32, name=f"p1{name}",
                                         tag="stp")
                    for idx, (dlo, dsz, dst) in enumerate(
                            ((0, P, dsta), (P, C - P, dstb))):
                        ps = slot[:, idx * TCH:(idx + 1) * TCH]
                        nc.tensor.matmul(
                            ps[0:dsz, :], wa[:, dlo:dlo + dsz],
                            xT_a[:, t0:t0 + TCH], start=True, stop=False)
                        nc.tensor.matmul(
                            ps[0:dsz, :], wb[:, dlo:dlo + dsz],
                            xT_b[:, t0:t0 + TCH], start=False, stop=True)
                        if name == "q":
                            nc.scalar.copy(dst[0:dsz, t0:t0 + TCH],
                                           ps[0:dsz, :])
                        else:
                            nc.vector.tensor_copy(
                                dst[0:dsz, t0:t0 + TCH], ps[0:dsz, :])
                slot = pst_pool.tile([P, 2 * TCH], F32, name="p1v", tag="stp")
                for i in range(NJ):
                    si = g * NJ + i
                    s0 = si * P
                    ps = slot[:, VOFF[i]:VOFF[i] + H * HS]
                    nc.tensor.matmul(ps, x16_a[:, s0:s0 + P],
                                     wv_a, start=True, stop=False)
                    nc.tensor.matmul(ps, x16_b[:, s0:s0 + P],
                                     wv_b, start=False, stop=True)
                    va_r = v33[si].rearrange("p (h e) -> p h e", h=H)
                    ps_r = ps.rearrange("p (h d) -> p h d", h=H)
                    nc.vector.tensor_copy(va_r[:, :, 0:HS], ps_r)
                    nc.gpsimd.memset(va_r[:, :, HS:HS + 1], 1.0)

            def emit_qkt(tci, si, p):
                tc0 = tci * TCH
                s0 = si * P
                stp = pst_pool.tile([P, 2 * TCH], F32, name="stp", tag="stp")
                for half in (0, 1):
                    h = 2 * p + half
                    kT_t, qT_t, pb = hsrc(h)
                    nc.tensor.matmul(
                        stp[:, half * TCH:(half + 1) * TCH],
                        kT_t[pb:pb + HS, s0:s0 + P],
                        qT_t[pb:pb + HS, tc0:tc0 + TCH],
                        start=True, stop=True, tile_position=(pb, 0))
                return stp

            def emit_exp(tci, si, p, stp):
                eng = _EXPPAT[si * 3 + p]
                ptp = ptp_pool.tile([P, 2 * TCH], BF16, name="ptp", tag="ptp")
                if eng == "A":
                    nc.scalar.activation(ptp, stp, Exp, scale=SCALE)
                else:
                    nc.vector.tensor_scalar(
                        ptp.bitcast(I16), stp, EXP_A, EXP_B,
                        mybir.AluOpType.mult, mybir.AluOpType.add)
                return ptp

            def emit_av(tci, si, ptps, heads=range(H)):
                av = state["av"]
                for h in heads:
                    bank, hl = divmod(h, 3)
                    ptp = ptps[h // 2]
                    half = h % 2
                    for j in range(NJ):
                        off = hl * 132 + j * 33
                        nc.tensor.matmul(
                            av[bank][:, off:off + 33],
                            ptp[:, half * TCH + j * P:half * TCH + (j + 1) * P],
                            v33[si][:, h * 33:(h + 1) * 33],
                            start=(si == 0 and hl == 0 and j == 0),
                            stop=(si == NS - 1 and hl == 2 and j == NJ - 1),
                            skip_group_check=True)

            def tail_norm(tci):
                av = state["av"]
                sa = stage_pool.tile([P, NJ * P], BF16, name="sa", tag="sa")
                sb = stage_pool.tile([P, NJ * (C - P)], BF16,
                                     name="sb", tag="sb")
                sa_v = sa.rearrange("p (j h e) -> p h j e", j=NJ, h=4)
                sb_v = sb.rearrange("p (j h e) -> p h j e", j=NJ, h=2)
                outs = []
                for bank in range(2):
                    rr = rr_pool.tile([P, 12], F32, name=f"rr{bank}",
                                      tag=f"rr{bank}")
                    av_v = av[bank][:, 0:396].rearrange(
                        "p (hl j e) -> p hl j e", hl=3, j=NJ)
                    rr_v = rr.rearrange("p (hl j e) -> p hl j e", hl=3, j=NJ)
                    nc.vector.reciprocal(rr_v, av_v[:, :, :, 32:33])
                    outs.append((av_v[:, :, :, 0:HS], rr_v))
                (o1, r1), (o2, r2) = outs
                for dst, src_, rsrc in (
                    (sa_v[:, 0:3], o1, r1),
                    (sa_v[:, 3:4], o2[:, 0:1], r2[:, 0:1]),
                    (sb_v[:, 0:2], o2[:, 1:3], r2[:, 1:3]),
                ):
                    s_b, r_b = broadcast_tensor_aps(src_, rsrc)
                    nc.vector.tensor_tensor(dst, s_b, r_b,
                                            mybir.AluOpType.mult)
                state["stage"] = (sa, sb)

            def tail_transpose(tci):
                sa, sb = state["stage"]
                slot = pst_pool.tile([P, 2 * TCH], F32, name="pot", tag="stp")
                pot = slot[:, 0:TCH].bitcast(BF16)
                for j in range(NJ):
                    nc.tensor.transpose(
                        pot[:, j * 2 * P:j * 2 * P + P],
                        sa[:, j * P:(j + 1) * P], id_sb)
                    nc.tensor.transpose(
                        pot[0:C - P, j * 2 * P + P:(j + 1) * 2 * P],
                        sb[:, j * (C - P):(j + 1) * (C - P)], id_sb)
                state["pot"] = pot

            def tail_otcopy(tci):
                pot = state["pot"]
                tc0 = tci * TCH
                pot_v = pot.rearrange("p (j two t) -> p j two t", j=NJ, two=2)
                dst_a = otn_a[:, tc0:tc0 + TCH].rearrange(
                    "p (j t) -> p j t", j=NJ)
                dst_b = otn_b[0:C - P, tc0:tc0 + TCH].rearrange(
                    "p (j t) -> p j t", j=NJ)
                nc.vector.tensor_copy(dst_a, pot_v[:, :, 0, :])
                nc.vector.tensor_copy(dst_b, pot_v[0:C - P, :, 1, :])

            def tail_py_alloc():
                state["py"] = pst_pool.tile([P, 2 * TCH], F32, name="pys",
                                            tag="stp")

            def tail_proj(tci, j):
                tt = tci * TCH + j * P
                py = state["py"][:, PYOFF[j]:PYOFF[j] + C]
                nc.tensor.matmul(py, otn_a[:, tt:tt + P], wpa_sb,
                                 start=True, stop=False)
                nc.tensor.matmul(py, otn_b[:, tt:tt + P], wpb_sb,
                                 start=False, stop=True)
                ysb = ysb_pool.tile([P, C], F32, name="ysb", tag="ysb")
                nc.vector.tensor_copy(ysb, py)
                nc.sync.dma_start(out[tt:tt + P, :], ysb)

            prev = None
            for tci in range(NT):
                av = [pav_pool.tile([P, TCH], F32, name=f"av{b}",
                                    tag=f"av{b}") for b in range(2)]
                state["av"] = av
                pend = None
                for si in range(NS):
                    if tci == 0 and si in (0, 1):
                        for g in (range(2) if si == 0 else range(2, 4)):
                            phase1_block(g)
                    ptps = []
                    for p in range(3):
                        if p == 2:
                            if pend is not None:
                                emit_av(tci, si - 1, pend, range(0, 3))
                        stp = emit_qkt(tci, si, p)
                        if p == 2 and pend is not None:
                            emit_av(tci, si - 1, pend, range(3, 6))
                            pend = None
                        ptps.append(emit_exp(tci, si, p, stp))
                        if prev is not None and si == 1 and p == 1:
                            tail_transpose(prev)
                    if prev is not None:
                        if si == 1:
                            tail_otcopy(prev)
                        elif si == 2:
                            tail_py_alloc()
                            tail_proj(prev, 0)
                            tail_proj(prev, 1)
                        elif si == 3:
                            tail_proj(prev, 2)
                            tail_proj(prev, 3)
                    pend = ptps
                emit_av(tci, NS - 1, pend)
                tail_norm(tci)
                prev = tci
            sa, sb = state["stage"]
            slot = pst_pool.tile([P, 2 * TCH], F32, name="potf", tag="stp")
            potf = slot[:, 0:TCH].bitcast(BF16)
            tail_py_alloc()
            tc0f = prev * TCH
            for j in range(NJ):
                nc.tensor.transpose(
                    potf[:, j * 2 * P:j * 2 * P + P],
                    sa[:, j * P:(j + 1) * P], id_sb)
                nc.tensor.transpose(
                    potf[0:C - P, j * 2 * P + P:(j + 1) * 2 * P],
                    sb[:, j * (C - P):(j + 1) * (C - P)], id_sb)
                tt = tc0f + j * P
                eng = nc.vector if j % 2 == 0 else nc.scalar
                if j % 2 == 0:
                    nc.vector.tensor_copy(otn_a[:, tt:tt + P],
                                          potf[:, j * 2 * P:j * 2 * P + P])
                    nc.vector.tensor_copy(
                        otn_b[0:C - P, tt:tt + P],
                        potf[0:C - P, j * 2 * P + P:(j + 1) * 2 * P])
                else:
                    nc.scalar.copy(otn_a[:, tt:tt + P],
                                   potf[:, j * 2 * P:j * 2 * P + P])
                    nc.scalar.copy(
                        otn_b[0:C - P, tt:tt + P],
                        potf[0:C - P, j * 2 * P + P:(j + 1) * 2 * P])
                py = state["py"][:, PYOFF[j]:PYOFF[j] + C]
                nc.tensor.matmul(py, otn_a[:, tt:tt + P], wpa_sb,
                                 start=True, stop=False)
                nc.tensor.matmul(py, otn_b[:, tt:tt + P], wpb_sb,
                                 start=False, stop=True)
                ysb = ysb_pool.tile([P, C], F32, name="ysbf", tag="ysb")
                if j % 2 == 0:
                    nc.scalar.copy(ysb, py)
                else:
                    nc.vector.tensor_copy(ysb, py)
                nc.sync.dma_start(out[tt:tt + P, :], ysb)

    nc.compile()
    return nc


def _get_nc():
    if "nc" not in _CACHE:
        _CACHE["nc"] = build_nc()
    return _CACHE["nc"]


def make_in_maps(x, Wq, Wk, Wv, Wproj, bproj):
    bf = ml_dtypes.bfloat16
    x = np.asarray(x, np.float32)
    pack32 = lambda w: np.ascontiguousarray(
        np.transpose(np.asarray(w, np.float32), (1, 0, 2)).reshape(C, H * HS))
    wq_, wk_ = pack32(Wq), pack32(Wk)
    wv_ = pack32(Wv).astype(bf)
    wp = np.asarray(Wproj, np.float32)
    wpa_ = np.ascontiguousarray(wp[0:P, :]).astype(bf)
    wpb_ = np.concatenate(
        [wp[P:, :], np.asarray(bproj, np.float32).reshape(1, C)], axis=0
    ).astype(bf)
    ident = np.eye(P, dtype=np.float32).astype(bf)
    maps = []
    for i in range(B):
        xti = np.ascontiguousarray(x[i].T)
        maps.append({"xT": xti, "x16": xti.astype(bf), "wq": wq_, "wk": wk_,
                     "wv": wv_, "wpa": wpa_, "wpb": wpb_, "ident": ident})
    return maps


def run(inputs, trace=False, **kw):
    nc = _get_nc()
    in_maps = make_in_maps(**inputs)
    res = run_bass_kernel_spmd(nc, in_maps, core_ids=list(range(B)),
                               trace=trace, **kw)
    y = np.stack([np.asarray(res.results[i]["out"], np.float32)
                  for i in range(B)], axis=0)
    return y, res


def kernel(**inputs):
    y, _ = run(inputs, trace=False)
    return y
